# revision 24
# baseline (speedup 1.0000x reference)
"""Trainium2 Bass kernel: cross-attention (4 heads, image->text) + GroupNorm.

Shapes (hardcoded): x [8, 512, 64, 64] f32, text_emb [8, 77, 768] f32,
attention_mask [8, 77] i32, Wq [512, 512], Wk/Wv [512, 768], biases [512],
gn_scale/bias [512]. Output [8, 512, 64, 64] f32.

Strategy: data-parallel over batch, one batch element per NeuronCore (8 cores).
Channels-on-partitions layout [C, S], S = H*W = 4096; fp16 16-bit dtype
everywhere on the PE/DVE paths (f32 PSUM accumulation).

Key restructure vs a direct translation: the Q projection is folded into the
score matmul via associativity:
    scores_h^T = Kh @ Qh^T = (Kh @ Wq_h) @ x = M_h @ x
so the big [C,S] Q tensor (and its PSUM->SBUF copies) never exists. M_h^T
([512, 77] per head) is computed on-chip from K^T (PE transpose) and Wq.
The Q-bias term folds into the exp() bias column (per-partition ACT bias).

Pipeline per head h (j = 8 chunks of 512 pixels):
  scores:  ps_s[77,512] = sum_t mstat[h,t]^T @ x_t[:,chunk]   (PE, fp16)
  exp:     eu = exp(SCALE*ps_s + SCALE*c_h)  ACT, PSUM->SBUF fp16
  denom:   ps_d[8,512] accumulates row j = maskf . eu(h,j)    (PE, mask8 trick)
  recip:   rb = exp(-ln(ps_d))                                 (ACT)
  bcast:   rb -> DRAM row [1,4096] -> one partition-broadcast DMA
           -> rbb[128,4096] (DMA is the only partition replicator)
  PV:      ps_u[128,512] = Vm_h^T @ eu                         (PE)
  norm:    y = ps_u * rbb                                      (DVE TT)
  stats:   sum(y) via DVE tensor_reduce, sum(y^2) via ACT Square
           with accum_out, per head                            (DVE+ACT)
  GN:      group sums via tiny matmuls, istd = exp(-0.5 ln(var+eps)),
           y*A + B -> fp16 staging -> one DMA per head (host upcasts)

All DRAM inputs are host-packed into [128, *] row-contiguous blocks so each
dma_start lowers to ~128 descriptors (descriptor count, not bytes, dominated
the DMA queues in earlier versions).
"""

import os
import numpy as np

NUM_HEADS = 4
GROUPS = 8
EPS = 1e-5
B, C, H, W = 8, 512, 64, 64
S = H * W          # 4096
L, E = 77, 768
DH = C // NUM_HEADS  # 128
N_CORES = 8
NCHUNK = 8         # S chunks of 512
CH = S // NCHUNK   # 512
SCALE = DH ** -0.5
GN_P = 64          # partitions per group
GN_N = float(GN_P * S)  # elements per group

_compiled = None


def _patch_tile_drain():
    """This container's walrus rejects multi-sem-wait Drain instructions
    ("Too many sync wait commands"); split the TileContext exit drain's waits
    into single-wait instructions, which lower like raw-bass waits."""
    import concourse.tile as tile
    import concourse.mybir as mybir
    from concourse.tile import ScopedClock

    if getattr(tile.TileContext, "_drain_patched", False):
        return

    def _patched(self, tick_clock, wait_clock):
        nc = self.nc
        probe = nc.sync.nop(nofuse=True, hint="drain_wait_probe")
        wait_clock.add_sem_waits(
            probe.ins, ScopedClock({None: tick_clock.global_clock})
        )
        si = probe.ins.sync_info
        waits = list(si.on_wait) if si is not None else []
        probe.ins.sync_info = mybir.SyncInfo(on_wait=[], on_update=[])

        popped = nc._tile_sem_poison_stack.pop()
        assert popped is self._sem_poison
        assert self.sems is not None
        allocated = self.sems.allocated()
        by_id = {h.num: h for h in allocated.values()}
        for wv in waits:
            h = by_id.get(wv.id)
            assert h is not None, f"no semaphore handle for wait {wv}"
            assert wv.wait_mode == "sem-ge-imm", wv
            nc.sync.wait_ge(h, wv.wait_value)

        nc.sync.drain()
        nc.all_engine_barrier()
        nc.clear_and_free_semaphores(list(allocated.values()))
        nc.all_engine_barrier()

    tile.TileContext._drain_and_barrier = _patched
    tile.TileContext._drain_patched = True


def _patch_ldw_opt():
    """Optionally flip walrus --enable-ldw-opt (env BASS_LDW_OPT=1)."""
    if os.environ.get("BASS_LDW_OPT") != "1":
        return
    import concourse.bass_utils as bu
    if getattr(bu, "_ldw_patched", False):
        return
    orig = bu.run_command

    def patched(cmd, *a, **kw):
        cmd = ["--enable-ldw-opt=true" if c == "--enable-ldw-opt=false" else c
               for c in cmd]
        return orig(cmd, *a, **kw)

    bu.run_command = patched
    bu._ldw_patched = True


def _split_multiwaits(nc):
    """This walrus build rejects instructions carrying more than one sem wait
    ("Too many sync wait commands"). Hoist all-but-one wait of each such
    instruction onto standalone event-semaphore waits (built by the real bass
    builders so they lower correctly), inserted just before it."""
    import concourse.mybir as mybir

    eng_map = {
        mybir.EngineType.DVE: nc.vector,
        mybir.EngineType.Activation: nc.scalar,
        mybir.EngineType.PE: nc.tensor,
        mybir.EngineType.Pool: nc.gpsimd,
        mybir.EngineType.SP: nc.sync,
    }
    jobs = []
    for f in nc.m.functions:
        for bb in f.blocks:
            for inst in bb.instructions:
                si = inst.sync_info
                if si is not None and len(si.on_wait) > 1:
                    jobs.append((bb, inst))
    if not jobs:
        return 0

    tail_bb = None
    made = []
    with nc.semaphore() as dummy:
        for bb, inst in jobs:
            waits = list(inst.sync_info.on_wait)
            for w in waits[:-1]:
                bi = eng_map[inst.engine].wait_ge(dummy, 0)
                bi.ins.sync_info = mybir.SyncInfo(on_wait=[w], on_update=[])
                made.append(bi.ins)
    made_names = {m.name for m in made}
    for f in nc.m.functions:
        for bb in f.blocks:
            if any(i.name in made_names for i in bb.instructions):
                tail_bb = bb
                tail_bb.instructions = [
                    i for i in bb.instructions if i.name not in made_names]
    assert tail_bb is not None

    it = iter(made)
    n_split = 0
    for f in nc.m.functions:
        for bb in f.blocks:
            out = []
            for inst in bb.instructions:
                si = inst.sync_info
                waits = list(si.on_wait) if si is not None else []
                if len(waits) > 1 and any(inst is j[1] for j in jobs):
                    for w in waits[:-1]:
                        ev = next(it)
                        out.append(ev)
                        n_split += 1
                    inst.sync_info = mybir.SyncInfo(
                        on_wait=[waits[-1]], on_update=list(si.on_update))
                out.append(inst)
            bb.instructions = out
    return n_split


def _build_nc():
    import concourse.bass as bass
    import concourse.tile as tile
    import concourse.mybir as mybir

    _patch_tile_drain()
    _patch_ldw_opt()
    dt = mybir.dt
    f32, fp16 = dt.float32, dt.float16
    AF = mybir.ActivationFunctionType
    OP = mybir.AluOpType
    AX = mybir.AxisListType

    nc = bass.Bass()

    # ---- DRAM I/O (host-packed, row-contiguous [128, *] blocks) ----
    d_xp = nc.dram_tensor("xp", [DH, 4 * S], fp16, kind="ExternalInput")
    d_ttp = nc.dram_tensor("ttp", [DH, 6 * L], fp16, kind="ExternalInput")
    d_wkp = nc.dram_tensor("wkp", [DH, 6 * CH], fp16, kind="ExternalInput")
    d_wvp = nc.dram_tensor("wvp", [DH, 6 * CH], fp16, kind="ExternalInput")
    d_wq = nc.dram_tensor("wq", [DH, 16 * DH + 4], fp16, kind="ExternalInput")
    d_lpk = nc.dram_tensor("lpk", [L, 141], fp16, kind="ExternalInput")
    d_maskf = nc.dram_tensor("maskf", [L, 1], f32, kind="ExternalInput")
    d_ones77 = nc.dram_tensor("ones77", [1, L], fp16, kind="ExternalInput")
    d_kvb = nc.dram_tensor("kvb", [1, 2 * C], fp16, kind="ExternalInput")
    d_gpk = nc.dram_tensor("gpk", [DH, 10], f32, kind="ExternalInput")
    d_gselT = nc.dram_tensor("gselT", [2, DH], f32, kind="ExternalInput")
    d_out = nc.dram_tensor("out", [C, S], fp16, kind="ExternalOutput")
    d_rbscr = nc.dram_tensor("rbscr", [NUM_HEADS, S], fp16, kind="Internal")

    ET = 6        # k tiles for E=768
    KT = 4        # c_in tiles for C=512

    from contextlib import ExitStack

    with tile.TileContext(nc) as tc, ExitStack() as stack:
        cpool = stack.enter_context(tc.tile_pool(name="const", bufs=1))
        t_lpk = cpool.tile([L, 141], fp16, tag="lpk")
        nc.sync.dma_start(out=t_lpk, in_=d_lpk.ap())
        t_maskf = cpool.tile([L, 1], f32, tag="maskf")
        nc.sync.dma_start(out=t_maskf, in_=d_maskf.ap())
        t_ones77 = cpool.tile([1, L], fp16, tag="ones77")
        nc.sync.dma_start(out=t_ones77, in_=d_ones77.ap())
        t_gpk = cpool.tile([DH, 10], f32, tag="gpk")
        nc.sync.dma_start(out=t_gpk, in_=d_gpk.ap())
        t_gselT = cpool.tile([2, DH], f32, tag="gselT")
        nc.sync.dma_start(out=t_gselT, in_=d_gselT.ap())
        t_wq = cpool.tile([DH, 16 * DH + 4], fp16, tag="wq")
        nc.sync.dma_start(out=t_wq, in_=d_wq.ap())
        t_eps = cpool.tile([2, 1], f32, tag="eps")
        nc.vector.memset(t_eps, EPS)

        t_mask8 = t_lpk[:, 0:64]
        t_ident = t_lpk[:, 64:141]
        t_gs4 = t_gpk[:, 0:4]
        t_gb4 = t_gpk[:, 4:8]
        t_gsel = t_gpk[:, 8:10]

        # persistent state
        t_kht = cpool.tile([DH, NUM_HEADS * L], fp16, tag="kht")
        t_vm = cpool.tile([L, C], fp16, tag="vm")
        t_mstat = cpool.tile([DH, 16 * L], fp16, tag="mstat")
        t_c4 = cpool.tile([L, 4], f32, tag="c4")
        t_eu = cpool.tile([L, NUM_HEADS * S], fp16, tag="eu")
        t_y = [cpool.tile([DH, S], fp16, tag=f"y{h}", name=f"y{h}")
               for h in range(NUM_HEADS)]
        t_rb = [cpool.tile([NCHUNK, CH], fp16, tag=f"rb{h}", name=f"rb{h}")
                for h in range(NUM_HEADS)]
        t_sq = cpool.tile([DH, S], fp16, tag="sq")
        t_syc = [cpool.tile([DH, NCHUNK], f32, tag=f"syc{h}", name=f"syc{h}")
                 for h in range(NUM_HEADS)]
        t_sy2 = [cpool.tile([DH, 2], f32, tag=f"sy2{h}", name=f"sy2{h}")
                 for h in range(NUM_HEADS)]
        t_xt = [cpool.tile([DH, S], fp16, tag=f"xt{t}", name=f"xt{t}")
                for t in range(KT)]

        # ---------- phase 0: K/V projections, K^T, M_h^T, c_h ----------
        with (
            tc.tile_pool(name="kvw", bufs=1) as kvw,
            tc.tile_pool(name="kvps", bufs=2, space="PSUM") as kvps,
        ):
            t_ttp = kvw.tile([DH, 6 * L], fp16, tag="ttp")
            nc.sync.dma_start(out=t_ttp, in_=d_ttp.ap())
            t_wkp = kvw.tile([DH, 6 * CH], fp16, tag="wkp")
            nc.sync.dma_start(out=t_wkp, in_=d_wkp.ap())
            t_kvb = kvw.tile([1, 2 * C], fp16, tag="kvb")
            nc.sync.dma_start(out=t_kvb, in_=d_kvb.ap())
            t_wvp = kvw.tile([DH, 6 * CH], fp16, tag="wvp")
            nc.sync.dma_start(out=t_wvp, in_=d_wvp.ap())

            # x DMAs after the phase-0 weights (HWDGE ring is FIFO; weights
            # gate the first matmuls)
            for t in range(KT):
                nc.sync.dma_start(out=t_xt[t][:, 0:1024],
                                  in_=d_xp.ap()[:, t * S:t * S + 1024])
            for t in range(KT):
                nc.sync.dma_start(out=t_xt[t][:, 1024:S],
                                  in_=d_xp.ap()[:, t * S + 1024:(t + 1) * S])

            # K^T per head DIRECTLY: kht[d, l] = sum_e Wk[128h+d, e]
            # * text[l, e] + bk  -- skips K-proj/transpose round trips
            for h in range(NUM_HEADS):
                ps_kh = kvps.tile([DH, L], f32, tag="pskh", name=f"pskh{h}")
                for k in range(ET):
                    nc.tensor.matmul(
                        ps_kh,
                        t_wkp[:, k * CH + h * DH:k * CH + (h + 1) * DH],
                        t_ttp[:, k * L:(k + 1) * L],
                        start=(k == 0), stop=False)
                nc.tensor.matmul(ps_kh, t_kvb[:, h * DH:(h + 1) * DH],
                                 t_ones77, start=False, stop=True)
                nc.vector.tensor_copy(out=t_kht[:, h * L:(h + 1) * L],
                                      in_=ps_kh)
                # M_h^T tiles: [c_tile 128, 77] per (h, t)
                ps_m = kvps.tile([DH, 4 * L], f32, tag="psm", name=f"psm{h}")
                for t in range(KT):
                    nc.tensor.matmul(
                        ps_m[:, t * L:(t + 1) * L],
                        t_wq[:, (h * 4 + t) * DH:(h * 4 + t + 1) * DH],
                        t_kht[:, h * L:(h + 1) * L],
                        start=True, stop=True)
                nc.vector.tensor_copy(
                    out=t_mstat[:, h * 4 * L:(h + 1) * 4 * L], in_=ps_m)

            # c_h = Kh @ bq_h  -> exp bias column (times SCALE)
            ps_c4 = kvps.tile([L, 4], f32, tag="psc4")
            for h in range(NUM_HEADS):
                nc.tensor.matmul(ps_c4[:, h:h + 1],
                                 t_kht[:, h * L:(h + 1) * L],
                                 t_wq[:, 16 * DH + h:16 * DH + h + 1],
                                 start=True, stop=True)
            nc.vector.tensor_scalar_mul(out=t_c4, in0=ps_c4, scalar1=SCALE)

            # V projection (only needed by the PV phase, ~15us later)
            ps_v = kvps.tile([L, C], f32, tag="pskv")
            for k in range(ET):
                nc.tensor.matmul(ps_v, t_ttp[:, k * L:(k + 1) * L],
                                 t_wvp[:, k * CH:(k + 1) * CH],
                                 start=(k == 0), stop=False)
            nc.tensor.matmul(ps_v, t_ones77, t_kvb[:, C:2 * C],
                             start=False, stop=True)
            nc.vector.tensor_scalar_mul(out=t_vm, in0=ps_v, scalar1=t_maskf)

        # ---------- main pipeline ----------
        spool = stack.enter_context(
            tc.tile_pool(name="spool", bufs=2, space="PSUM"))
        dpool = stack.enter_context(
            tc.tile_pool(name="dpool", bufs=1, space="PSUM"))
        upool = stack.enter_context(
            tc.tile_pool(name="upool", bufs=2, space="PSUM"))
        gnps = stack.enter_context(
            tc.tile_pool(name="gnps", bufs=1, space="PSUM"))
        rbbpool = stack.enter_context(tc.tile_pool(name="rbb", bufs=2))
        lnpool = stack.enter_context(tc.tile_pool(name="lnp", bufs=2))
        stgpool = stack.enter_context(tc.tile_pool(name="stg", bufs=2))
        gnsb = stack.enter_context(tc.tile_pool(name="gnsb", bufs=1))

        rbb_tiles = {}

        def S_block(h, jp0, jp1):
            for jp in range(jp0, jp1):
                ps_s = spool.tile([L, 2 * CH], f32, tag="pss",
                                  name=f"pss{h}_{jp}")
                for u in range(2):
                    for t in range(KT):
                        nc.tensor.matmul(
                            ps_s[:, u * CH:(u + 1) * CH],
                            t_mstat[:, (h * 4 + t) * L:(h * 4 + t + 1) * L],
                            t_xt[t][:, (2 * jp + u) * CH:(2 * jp + u + 1) * CH],
                            start=(t == 0), stop=(t == KT - 1))
                nc.scalar.activation(
                    out=t_eu[:, (h * NCHUNK + 2 * jp) * CH:
                             (h * NCHUNK + 2 * jp + 2) * CH],
                    in_=ps_s, func=AF.Exp, scale=SCALE,
                    bias=t_c4[:, h:h + 1])

        def D_block(h):
            ps_d = dpool.tile([NCHUNK, CH], f32, tag="psd", name=f"psd{h}")
            for j in range(NCHUNK):
                nc.tensor.matmul(
                    ps_d,
                    t_mask8[:, j * NCHUNK:(j + 1) * NCHUNK],
                    t_eu[:, (h * NCHUNK + j) * CH:(h * NCHUNK + j + 1) * CH],
                    start=(j == 0), stop=(j == NCHUNK - 1))
            t_ln = lnpool.tile([NCHUNK, CH], f32, tag="lnd", name=f"lnd{h}")
            nc.scalar.activation(out=t_ln, in_=ps_d, func=AF.Ln)
            nc.scalar.activation(out=t_rb[h], in_=t_ln, func=AF.Exp,
                                 scale=-1.0)
            nc.sync.dma_start(
                out=d_rbscr.ap()[h:h + 1, :]
                .rearrange("r (p f) -> (r p) f", p=NCHUNK),
                in_=t_rb[h])
            rbb = rbbpool.tile([DH, S], fp16, tag="rbb", name=f"rbb{h}")
            nc.sync.dma_start(
                out=rbb.rearrange("p (one f) -> p one f", one=1),
                in_=d_rbscr.ap()[h:h + 1, :].partition_broadcast(DH))
            rbb_tiles[h] = rbb

        def P_block(h, j0, j1):
            for j in range(j0, j1):
                ps_u = upool.tile([DH, CH], f32, tag="psu",
                                  name=f"psu{h}_{j}")
                nc.tensor.matmul(
                    ps_u,
                    t_vm[:, h * DH:(h + 1) * DH],
                    t_eu[:, (h * NCHUNK + j) * CH:(h * NCHUNK + j + 1) * CH],
                    start=True, stop=True)
                nc.vector.scalar_tensor_tensor(
                    out=t_y[h][:, j * CH:(j + 1) * CH], in0=ps_u,
                    scalar=1.0,
                    in1=rbb_tiles[h][:, j * CH:(j + 1) * CH],
                    op0=OP.mult, op1=OP.mult,
                    accum_out=t_syc[h][:, j:j + 1])


        def SQ_block(h, half):
            hs = S // 2
            nc.scalar.activation(
                out=t_sq[:, half * hs:(half + 1) * hs],
                in_=t_y[h][:, half * hs:(half + 1) * hs], func=AF.Square,
                accum_out=t_sy2[h][:, half:half + 1])

        def G_block(h):
            st3 = gnsb.tile([DH, 3], f32, tag=f"st3{h}", name=f"st3{h}")
            nc.vector.tensor_reduce(out=st3[:, 0:1], in_=t_syc[h],
                                    axis=AX.X, op=OP.add)
            nc.vector.tensor_copy(out=st3[:, 1:3], in_=t_sy2[h])
            ps_gn = gnps.tile([DH, 8], f32, tag="psgn", name=f"psgn{h}")
            nc.tensor.matmul(ps_gn[0:2, 0:3], t_gsel, st3,
                             start=True, stop=True)
            t_mE = gnsb.tile([2, 3], f32, tag=f"mE{h}", name=f"mE{h}")
            nc.vector.tensor_scalar_mul(out=t_mE, in0=ps_gn[0:2, 0:3],
                                        scalar1=1.0 / GN_N)
            t_m2 = gnsb.tile([2, 1], f32, tag=f"m2_{h}", name=f"m2_{h}")
            nc.vector.tensor_scalar_mul(out=t_m2, in0=t_mE[:, 0:1],
                                        scalar1=t_mE[:, 0:1])
            t_var = gnsb.tile([2, 1], f32, tag=f"var{h}", name=f"var{h}")
            nc.vector.tensor_add(out=t_var, in0=t_mE[:, 1:2],
                                 in1=t_mE[:, 2:3])
            nc.vector.tensor_sub(out=t_var, in0=t_var, in1=t_m2)
            t_lnv = gnsb.tile([2, 1], f32, tag=f"lnv{h}", name=f"lnv{h}")
            nc.scalar.activation(out=t_lnv, in_=t_var, func=AF.Ln, bias=t_eps)
            t_im = gnsb.tile([2, 2], f32, tag=f"im{h}", name=f"im{h}")
            nc.scalar.activation(out=t_im[:, 0:1], in_=t_lnv, func=AF.Exp,
                                 scale=-0.5)
            nc.vector.tensor_copy(out=t_im[:, 1:2], in_=t_mE[:, 0:1])
            nc.tensor.matmul(ps_gn[:, 4:6], t_gselT, t_im,
                             start=True, stop=True)
            t_A = gnsb.tile([DH, 1], f32, tag=f"A{h}", name=f"A{h}")
            nc.vector.tensor_mul(out=t_A, in0=ps_gn[:, 4:5],
                                 in1=t_gs4[:, h:h + 1])
            t_t1 = gnsb.tile([DH, 1], f32, tag=f"t1_{h}", name=f"t1_{h}")
            nc.vector.tensor_mul(out=t_t1, in0=ps_gn[:, 5:6], in1=t_A)
            t_B = gnsb.tile([DH, 1], f32, tag=f"B{h}", name=f"B{h}")
            nc.vector.tensor_sub(out=t_B, in0=t_gb4[:, h:h + 1], in1=t_t1)
            stg = stgpool.tile([DH, S], fp16, tag="stg", name=f"stg{h}")
            nc.vector.tensor_scalar(
                out=stg, in0=t_y[h],
                scalar1=t_A, scalar2=t_B, op0=OP.mult, op1=OP.add)
            nc.sync.dma_start(out=d_out.ap()[h * DH:(h + 1) * DH, :], in_=stg)

        # interleaved emission: per-engine FIFO order == emission order
        S_block(0, 0, 4)
        S_block(1, 0, 1)
        D_block(0)
        S_block(1, 1, 4)
        S_block(2, 0, 1)
        D_block(1)
        P_block(0, 0, 4)
        SQ_block(0, 0)
        P_block(0, 4, 8)
        S_block(2, 1, 4)
        SQ_block(0, 1)
        S_block(3, 0, 1)
        D_block(2)
        P_block(1, 0, 4)
        SQ_block(1, 0)
        G_block(0)
        S_block(3, 1, 2)
        P_block(1, 4, 8)
        SQ_block(1, 1)
        S_block(3, 2, 4)
        P_block(2, 0, 4)
        SQ_block(2, 0)
        D_block(3)
        P_block(2, 4, 8)
        SQ_block(2, 1)
        G_block(1)
        P_block(3, 0, 4)
        SQ_block(3, 0)
        P_block(3, 4, 8)
        SQ_block(3, 1)
        G_block(2)
        G_block(3)

    _split_multiwaits(nc)
    return nc


def _prepare_in_maps(x, text_emb, attention_mask, Wq_w, Wq_b, Wk_w, Wk_b,
                     Wv_w, Wv_b, gn_scale, gn_bias):
    f32 = np.float32
    fp16 = np.float16
    x = np.asarray(x)
    text_emb = np.asarray(text_emb)
    attention_mask = np.asarray(attention_mask)
    Wq_w = np.asarray(Wq_w)
    Wq_b = np.asarray(Wq_b)
    Wk_w = np.asarray(Wk_w)
    Wk_b = np.asarray(Wk_b)
    Wv_w = np.asarray(Wv_w)
    Wv_b = np.asarray(Wv_b)
    gn_scale = np.asarray(gn_scale)
    gn_bias = np.asarray(gn_bias)

    wkT = np.ascontiguousarray(Wk_w.T).reshape(6, DH, C)
    wvT = np.ascontiguousarray(Wv_w.T).reshape(6, DH, C)
    wkp = np.ascontiguousarray(wkT.transpose(1, 0, 2).reshape(DH, 6 * CH)
                               ).astype(fp16)
    wvp = np.ascontiguousarray(wvT.transpose(1, 0, 2).reshape(DH, 6 * CH)
                               ).astype(fp16)

    # wq[d, (h*4+t)*128 + c] = Wq[128h + d, 128t + c]; last 4 cols = Wq_b
    wqr = Wq_w.reshape(NUM_HEADS, DH, 4, DH)
    wq = np.empty((DH, 16 * DH + 4), fp16)
    wq[:, 0:16 * DH] = wqr.transpose(1, 0, 2, 3).reshape(DH, 16 * DH)
    wq[:, 16 * DH:] = Wq_b.reshape(4, DH).T

    kvb = np.empty((1, 2 * C), fp16)
    kvb[0, 0:C] = Wk_b
    kvb[0, C:2 * C] = Wv_b

    gpk = np.zeros((DH, 10), f32)
    gpk[:, 0:4] = gn_scale.reshape(4, DH).T
    gpk[:, 4:8] = gn_bias.reshape(4, DH).T
    gpk[0:64, 8] = 1.0
    gpk[64:128, 9] = 1.0
    gselT = np.ascontiguousarray(gpk[:, 8:10].T)

    ones77 = np.ones((1, L), fp16)

    xp = np.ascontiguousarray(
        x.reshape(B, 4, DH, S).transpose(0, 2, 1, 3).reshape(B, DH, 4 * S)
    ).astype(fp16)
    ttp = np.ascontiguousarray(
        text_emb.transpose(0, 2, 1).reshape(B, 6, DH, L)
        .transpose(0, 2, 1, 3).reshape(B, DH, 6 * L)).astype(fp16)

    in_maps = []
    for b in range(N_CORES):
        maskf = attention_mask[b].astype(f32)
        lpk = np.zeros((L, 141), fp16)
        for j in range(NCHUNK):
            lpk[:, j * NCHUNK + j] = maskf
        lpk[:, 64:141] = np.eye(L, dtype=fp16)
        in_maps.append({
            "xp": xp[b],
            "ttp": ttp[b],
            "wkp": wkp, "wvp": wvp,
            "wq": wq,
            "lpk": lpk,
            "maskf": maskf.reshape(L, 1),
            "ones77": ones77,
            "kvb": kvb,
            "gpk": gpk,
            "gselT": gselT,
        })
    return in_maps


def kernel(**inputs):
    global _compiled
    from concourse import bass_utils

    in_maps = _prepare_in_maps(**inputs)
    if _compiled is None:
        _compiled = _build_nc()
    res = bass_utils.run_bass_kernel_spmd(
        _compiled, in_maps, core_ids=list(range(N_CORES)))
    out = np.stack([np.asarray(res.results[b]["out"]).astype(np.float32)
                    .reshape(C, H, W) for b in range(N_CORES)])
    return out


# revision 26
# speedup vs baseline: 1.1326x; 1.1326x over previous
"""Trainium2 Bass kernel: cross-attention (4 heads, image->text) + GroupNorm.

Shapes (hardcoded): x [8, 512, 64, 64] f32, text_emb [8, 77, 768] f32,
attention_mask [8, 77] i32, Wq [512, 512], Wk/Wv [512, 768], biases [512],
gn_scale/bias [512]. Output [8, 512, 64, 64] f32.

Strategy: data-parallel over batch, one batch element per NeuronCore (8 cores).
Channels-on-partitions layout [C, S], S = H*W = 4096; fp16 16-bit dtype
everywhere on the PE/DVE paths (f32 PSUM accumulation).

Key restructure vs a direct translation: the Q projection is folded into the
score matmul via associativity:
    scores_h^T = Kh @ Qh^T = (Kh @ Wq_h) @ x = M_h @ x
so the big [C,S] Q tensor (and its PSUM->SBUF copies) never exists. M_h^T
([512, 77] per head) is computed on-chip from K^T (PE transpose) and Wq.
The Q-bias term folds into the exp() bias column (per-partition ACT bias).

Pipeline per head h (j = 8 chunks of 512 pixels):
  scores:  ps_s[77,512] = sum_t mstat[h,t]^T @ x_t[:,chunk]   (PE, fp16)
  exp:     eu = exp(SCALE*ps_s + SCALE*c_h)  ACT, PSUM->SBUF fp16
  denom:   ps_d[8,512] accumulates row j = maskf . eu(h,j)    (PE, mask8 trick)
  recip:   rb = exp(-ln(ps_d))                                 (ACT)
  bcast:   rb -> DRAM row [1,4096] -> one partition-broadcast DMA
           -> rbb[128,4096] (DMA is the only partition replicator)
  PV:      ps_u[128,512] = Vm_h^T @ eu                         (PE)
  norm:    y = ps_u * rbb                                      (DVE TT)
  stats:   sum(y) via DVE tensor_reduce, sum(y^2) via ACT Square
           with accum_out, per head                            (DVE+ACT)
  GN:      group sums via tiny matmuls, istd = exp(-0.5 ln(var+eps)),
           y*A + B -> fp16 staging -> one DMA per head (host upcasts)

All DRAM inputs are host-packed into [128, *] row-contiguous blocks so each
dma_start lowers to ~128 descriptors (descriptor count, not bytes, dominated
the DMA queues in earlier versions).
"""

import os
import numpy as np

NUM_HEADS = 4
GROUPS = 8
EPS = 1e-5
B, C, H, W = 8, 512, 64, 64
S = H * W          # 4096
L, E = 77, 768
DH = C // NUM_HEADS  # 128
N_CORES = 8
NCHUNK = 8         # S chunks of 512
CH = S // NCHUNK   # 512
SCALE = DH ** -0.5
GN_P = 64          # partitions per group
GN_N = float(GN_P * S)  # elements per group

_compiled = None


def _patch_tile_drain():
    """This container's walrus rejects multi-sem-wait Drain instructions
    ("Too many sync wait commands"); split the TileContext exit drain's waits
    into single-wait instructions, which lower like raw-bass waits."""
    import concourse.tile as tile
    import concourse.mybir as mybir
    from concourse.tile import ScopedClock

    if getattr(tile.TileContext, "_drain_patched", False):
        return

    def _patched(self, tick_clock, wait_clock):
        nc = self.nc
        probe = nc.sync.nop(nofuse=True, hint="drain_wait_probe")
        wait_clock.add_sem_waits(
            probe.ins, ScopedClock({None: tick_clock.global_clock})
        )
        si = probe.ins.sync_info
        waits = list(si.on_wait) if si is not None else []
        probe.ins.sync_info = mybir.SyncInfo(on_wait=[], on_update=[])

        popped = nc._tile_sem_poison_stack.pop()
        assert popped is self._sem_poison
        assert self.sems is not None
        allocated = self.sems.allocated()
        by_id = {h.num: h for h in allocated.values()}
        for wv in waits:
            h = by_id.get(wv.id)
            assert h is not None, f"no semaphore handle for wait {wv}"
            assert wv.wait_mode == "sem-ge-imm", wv
            nc.sync.wait_ge(h, wv.wait_value)

        nc.sync.drain()
        nc.all_engine_barrier()
        nc.clear_and_free_semaphores(list(allocated.values()))
        nc.all_engine_barrier()

    tile.TileContext._drain_and_barrier = _patched
    tile.TileContext._drain_patched = True


def _patch_ldw_opt():
    """Optionally flip walrus --enable-ldw-opt (env BASS_LDW_OPT=1)."""
    if os.environ.get("BASS_LDW_OPT") != "1":
        return
    import concourse.bass_utils as bu
    if getattr(bu, "_ldw_patched", False):
        return
    orig = bu.run_command

    def patched(cmd, *a, **kw):
        cmd = ["--enable-ldw-opt=true" if c == "--enable-ldw-opt=false" else c
               for c in cmd]
        return orig(cmd, *a, **kw)

    bu.run_command = patched
    bu._ldw_patched = True


def _split_multiwaits(nc):
    """This walrus build rejects instructions carrying more than one sem wait
    ("Too many sync wait commands"). Hoist all-but-one wait of each such
    instruction onto standalone event-semaphore waits (built by the real bass
    builders so they lower correctly), inserted just before it."""
    import concourse.mybir as mybir

    eng_map = {
        mybir.EngineType.DVE: nc.vector,
        mybir.EngineType.Activation: nc.scalar,
        mybir.EngineType.PE: nc.tensor,
        mybir.EngineType.Pool: nc.gpsimd,
        mybir.EngineType.SP: nc.sync,
    }
    jobs = []
    for f in nc.m.functions:
        for bb in f.blocks:
            for inst in bb.instructions:
                si = inst.sync_info
                if si is not None and len(si.on_wait) > 1:
                    jobs.append((bb, inst))
    if not jobs:
        return 0

    tail_bb = None
    made = []
    with nc.semaphore() as dummy:
        for bb, inst in jobs:
            waits = list(inst.sync_info.on_wait)
            for w in waits[:-1]:
                bi = eng_map[inst.engine].wait_ge(dummy, 0)
                bi.ins.sync_info = mybir.SyncInfo(on_wait=[w], on_update=[])
                made.append(bi.ins)
    made_names = {m.name for m in made}
    for f in nc.m.functions:
        for bb in f.blocks:
            if any(i.name in made_names for i in bb.instructions):
                tail_bb = bb
                tail_bb.instructions = [
                    i for i in bb.instructions if i.name not in made_names]
    assert tail_bb is not None

    it = iter(made)
    n_split = 0
    for f in nc.m.functions:
        for bb in f.blocks:
            out = []
            for inst in bb.instructions:
                si = inst.sync_info
                waits = list(si.on_wait) if si is not None else []
                if len(waits) > 1 and any(inst is j[1] for j in jobs):
                    for w in waits[:-1]:
                        ev = next(it)
                        out.append(ev)
                        n_split += 1
                    inst.sync_info = mybir.SyncInfo(
                        on_wait=[waits[-1]], on_update=list(si.on_update))
                out.append(inst)
            bb.instructions = out
    return n_split


def _build_nc():
    import concourse.bass as bass
    import concourse.tile as tile
    import concourse.mybir as mybir

    _patch_tile_drain()
    _patch_ldw_opt()
    dt = mybir.dt
    f32, fp16 = dt.float32, dt.float16
    AF = mybir.ActivationFunctionType
    OP = mybir.AluOpType
    AX = mybir.AxisListType

    nc = bass.Bass()

    # ---- DRAM I/O (host-packed, row-contiguous [128, *] blocks) ----
    d_xp = nc.dram_tensor("xp", [DH, 4 * S], fp16, kind="ExternalInput")
    d_ttp = nc.dram_tensor("ttp", [DH, 6 * L], fp16, kind="ExternalInput")
    d_wkp = nc.dram_tensor("wkp", [DH, 6 * CH], fp16, kind="ExternalInput")
    d_wvp = nc.dram_tensor("wvp", [DH, 6 * CH], fp16, kind="ExternalInput")
    d_wq = nc.dram_tensor("wq", [DH, 16 * DH + 4], fp16, kind="ExternalInput")
    d_lpk = nc.dram_tensor("lpk", [L, 141], fp16, kind="ExternalInput")
    d_maskf = nc.dram_tensor("maskf", [L, 1], f32, kind="ExternalInput")
    d_ones77 = nc.dram_tensor("ones77", [1, L], fp16, kind="ExternalInput")
    d_kvb = nc.dram_tensor("kvb", [1, 2 * C], fp16, kind="ExternalInput")
    d_gpk = nc.dram_tensor("gpk", [DH, 10], f32, kind="ExternalInput")
    d_gselT = nc.dram_tensor("gselT", [2, DH], f32, kind="ExternalInput")
    d_out = nc.dram_tensor("out", [C, S], fp16, kind="ExternalOutput")
    d_rbscr = nc.dram_tensor("rbscr", [NUM_HEADS, S], fp16, kind="Internal")

    ET = 6        # k tiles for E=768
    KT = 4        # c_in tiles for C=512

    from contextlib import ExitStack

    with tile.TileContext(nc) as tc, ExitStack() as stack:
        cpool = stack.enter_context(tc.tile_pool(name="const", bufs=1))
        t_lpk = cpool.tile([L, 141], fp16, tag="lpk")
        nc.sync.dma_start(out=t_lpk, in_=d_lpk.ap())
        t_maskf = cpool.tile([L, 1], f32, tag="maskf")
        nc.sync.dma_start(out=t_maskf, in_=d_maskf.ap())
        t_ones77 = cpool.tile([1, L], fp16, tag="ones77")
        nc.sync.dma_start(out=t_ones77, in_=d_ones77.ap())
        t_gpk = cpool.tile([DH, 10], f32, tag="gpk")
        nc.sync.dma_start(out=t_gpk, in_=d_gpk.ap())
        t_gselT = cpool.tile([2, DH], f32, tag="gselT")
        nc.sync.dma_start(out=t_gselT, in_=d_gselT.ap())
        t_wq = cpool.tile([DH, 16 * DH + 4], fp16, tag="wq")
        nc.sync.dma_start(out=t_wq, in_=d_wq.ap())
        t_eps = cpool.tile([2, 1], f32, tag="eps")
        nc.vector.memset(t_eps, EPS)

        t_mask8 = t_lpk[:, 0:64]
        t_ident = t_lpk[:, 64:141]
        t_gs4 = t_gpk[:, 0:4]
        t_gb4 = t_gpk[:, 4:8]
        t_gsel = t_gpk[:, 8:10]

        # persistent state
        t_kht = cpool.tile([DH, NUM_HEADS * L], fp16, tag="kht")
        t_vm = cpool.tile([L, C], fp16, tag="vm")
        t_mstat = cpool.tile([DH, 16 * L], fp16, tag="mstat")
        t_c4 = cpool.tile([L, 4], f32, tag="c4")
        t_eu = cpool.tile([L, NUM_HEADS * S], fp16, tag="eu")
        t_y = [cpool.tile([DH, S], fp16, tag=f"y{h}", name=f"y{h}")
               for h in range(NUM_HEADS)]
        t_rb = [cpool.tile([NCHUNK, CH], fp16, tag=f"rb{h}", name=f"rb{h}")
                for h in range(NUM_HEADS)]
        t_sq = cpool.tile([DH, S], fp16, tag="sq")
        t_syc = [cpool.tile([DH, NCHUNK], f32, tag=f"syc{h}", name=f"syc{h}")
                 for h in range(NUM_HEADS)]
        t_sy2 = [cpool.tile([DH, 2], f32, tag=f"sy2{h}", name=f"sy2{h}")
                 for h in range(NUM_HEADS)]
        t_xt = [cpool.tile([DH, S], fp16, tag=f"xt{t}", name=f"xt{t}")
                for t in range(KT)]

        # ---------- phase 0: K/V projections, K^T, M_h^T, c_h ----------
        with (
            tc.tile_pool(name="kvw", bufs=1) as kvw,
            tc.tile_pool(name="kvps", bufs=2, space="PSUM") as kvps,
        ):
            t_ttp = kvw.tile([DH, 6 * L], fp16, tag="ttp")
            nc.sync.dma_start(out=t_ttp, in_=d_ttp.ap())
            t_wkp = kvw.tile([DH, 6 * CH], fp16, tag="wkp")
            nc.sync.dma_start(out=t_wkp, in_=d_wkp.ap())
            t_kvb = kvw.tile([1, 2 * C], fp16, tag="kvb")
            nc.sync.dma_start(out=t_kvb, in_=d_kvb.ap())

            # x first chunks before the V weights: x gates the first score
            # matmuls (~16us in); Vm isn't consumed until the PV phase
            for t in range(KT):
                nc.sync.dma_start(out=t_xt[t][:, 0:1024],
                                  in_=d_xp.ap()[:, t * S:t * S + 1024])
            t_wvp = kvw.tile([DH, 6 * CH], fp16, tag="wvp")
            nc.sync.dma_start(out=t_wvp, in_=d_wvp.ap())
            for t in range(KT):
                nc.sync.dma_start(out=t_xt[t][:, 1024:S],
                                  in_=d_xp.ap()[:, t * S + 1024:(t + 1) * S])

            # K^T per head DIRECTLY: kht[d, l] = sum_e Wk[128h+d, e]
            # * text[l, e] + bk  -- skips K-proj/transpose round trips
            for h in range(NUM_HEADS):
                ps_kh = kvps.tile([DH, L], f32, tag="pskh", name=f"pskh{h}")
                for k in range(ET):
                    nc.tensor.matmul(
                        ps_kh,
                        t_wkp[:, k * CH + h * DH:k * CH + (h + 1) * DH],
                        t_ttp[:, k * L:(k + 1) * L],
                        start=(k == 0), stop=False)
                nc.tensor.matmul(ps_kh, t_kvb[:, h * DH:(h + 1) * DH],
                                 t_ones77, start=False, stop=True)
                nc.vector.tensor_copy(out=t_kht[:, h * L:(h + 1) * L],
                                      in_=ps_kh)
                # M_h^T tiles: [c_tile 128, 77] per (h, t)
                ps_m = kvps.tile([DH, 4 * L], f32, tag="psm", name=f"psm{h}")
                for t in range(KT):
                    nc.tensor.matmul(
                        ps_m[:, t * L:(t + 1) * L],
                        t_wq[:, (h * 4 + t) * DH:(h * 4 + t + 1) * DH],
                        t_kht[:, h * L:(h + 1) * L],
                        start=True, stop=True)
                nc.vector.tensor_copy(
                    out=t_mstat[:, h * 4 * L:(h + 1) * 4 * L], in_=ps_m)

            # c_h = Kh @ bq_h  -> exp bias column (times SCALE)
            ps_c4 = kvps.tile([L, 4], f32, tag="psc4")
            for h in range(NUM_HEADS):
                nc.tensor.matmul(ps_c4[:, h:h + 1],
                                 t_kht[:, h * L:(h + 1) * L],
                                 t_wq[:, 16 * DH + h:16 * DH + h + 1],
                                 start=True, stop=True)
            nc.vector.tensor_scalar_mul(out=t_c4, in0=ps_c4, scalar1=SCALE)

            # V projection (only needed by the PV phase, ~15us later)
            ps_v = kvps.tile([L, C], f32, tag="pskv")
            for k in range(ET):
                nc.tensor.matmul(ps_v, t_ttp[:, k * L:(k + 1) * L],
                                 t_wvp[:, k * CH:(k + 1) * CH],
                                 start=(k == 0), stop=False)
            nc.tensor.matmul(ps_v, t_ones77, t_kvb[:, C:2 * C],
                             start=False, stop=True)
            nc.vector.tensor_scalar_mul(out=t_vm, in0=ps_v, scalar1=t_maskf)

        # ---------- main pipeline ----------
        spool = stack.enter_context(
            tc.tile_pool(name="spool", bufs=2, space="PSUM"))
        dpool = stack.enter_context(
            tc.tile_pool(name="dpool", bufs=1, space="PSUM"))
        upool = stack.enter_context(
            tc.tile_pool(name="upool", bufs=2, space="PSUM"))
        gnps = stack.enter_context(
            tc.tile_pool(name="gnps", bufs=1, space="PSUM"))
        rbbpool = stack.enter_context(tc.tile_pool(name="rbb", bufs=2))
        lnpool = stack.enter_context(tc.tile_pool(name="lnp", bufs=2))
        stgpool = stack.enter_context(tc.tile_pool(name="stg", bufs=2))
        gnsb = stack.enter_context(tc.tile_pool(name="gnsb", bufs=1))

        rbb_tiles = {}

        def S_block(h, jp0, jp1):
            for jp in range(jp0, jp1):
                ps_s = spool.tile([L, 2 * CH], f32, tag="pss",
                                  name=f"pss{h}_{jp}")
                for u in range(2):
                    for t in range(KT):
                        nc.tensor.matmul(
                            ps_s[:, u * CH:(u + 1) * CH],
                            t_mstat[:, (h * 4 + t) * L:(h * 4 + t + 1) * L],
                            t_xt[t][:, (2 * jp + u) * CH:(2 * jp + u + 1) * CH],
                            start=(t == 0), stop=(t == KT - 1))
                nc.scalar.activation(
                    out=t_eu[:, (h * NCHUNK + 2 * jp) * CH:
                             (h * NCHUNK + 2 * jp + 2) * CH],
                    in_=ps_s, func=AF.Exp, scale=SCALE,
                    bias=t_c4[:, h:h + 1])

        def D_block(h):
            ps_d = dpool.tile([NCHUNK, CH], f32, tag="psd", name=f"psd{h}")
            for j in range(NCHUNK):
                nc.tensor.matmul(
                    ps_d,
                    t_mask8[:, j * NCHUNK:(j + 1) * NCHUNK],
                    t_eu[:, (h * NCHUNK + j) * CH:(h * NCHUNK + j + 1) * CH],
                    start=(j == 0), stop=(j == NCHUNK - 1))
            t_ln = lnpool.tile([NCHUNK, CH], f32, tag="lnd", name=f"lnd{h}")
            nc.scalar.activation(out=t_ln, in_=ps_d, func=AF.Ln)
            nc.scalar.activation(out=t_rb[h], in_=t_ln, func=AF.Exp,
                                 scale=-1.0)
            nc.sync.dma_start(
                out=d_rbscr.ap()[h:h + 1, :]
                .rearrange("r (p f) -> (r p) f", p=NCHUNK),
                in_=t_rb[h])
            rbb = rbbpool.tile([DH, S], fp16, tag="rbb", name=f"rbb{h}")
            nc.sync.dma_start(
                out=rbb.rearrange("p (one f) -> p one f", one=1),
                in_=d_rbscr.ap()[h:h + 1, :].partition_broadcast(DH))
            rbb_tiles[h] = rbb

        def P_block(h, j0, j1):
            for j in range(j0, j1):
                ps_u = upool.tile([DH, CH], f32, tag="psu",
                                  name=f"psu{h}_{j}")
                nc.tensor.matmul(
                    ps_u,
                    t_vm[:, h * DH:(h + 1) * DH],
                    t_eu[:, (h * NCHUNK + j) * CH:(h * NCHUNK + j + 1) * CH],
                    start=True, stop=True)
                nc.vector.scalar_tensor_tensor(
                    out=t_y[h][:, j * CH:(j + 1) * CH], in0=ps_u,
                    scalar=1.0,
                    in1=rbb_tiles[h][:, j * CH:(j + 1) * CH],
                    op0=OP.mult, op1=OP.mult,
                    accum_out=t_syc[h][:, j:j + 1])


        def SQ_block(h, half):
            hs = S // 2
            nc.scalar.activation(
                out=t_sq[:, half * hs:(half + 1) * hs],
                in_=t_y[h][:, half * hs:(half + 1) * hs], func=AF.Square,
                accum_out=t_sy2[h][:, half:half + 1])

        def G_block(h):
            st3 = gnsb.tile([DH, 3], f32, tag=f"st3{h}", name=f"st3{h}")
            nc.vector.tensor_reduce(out=st3[:, 0:1], in_=t_syc[h],
                                    axis=AX.X, op=OP.add)
            nc.vector.tensor_copy(out=st3[:, 1:3], in_=t_sy2[h])
            ps_gn = gnps.tile([DH, 8], f32, tag="psgn", name=f"psgn{h}")
            nc.tensor.matmul(ps_gn[0:2, 0:3], t_gsel, st3,
                             start=True, stop=True)
            t_mE = gnsb.tile([2, 3], f32, tag=f"mE{h}", name=f"mE{h}")
            nc.vector.tensor_scalar_mul(out=t_mE, in0=ps_gn[0:2, 0:3],
                                        scalar1=1.0 / GN_N)
            t_m2 = gnsb.tile([2, 1], f32, tag=f"m2_{h}", name=f"m2_{h}")
            nc.vector.tensor_scalar_mul(out=t_m2, in0=t_mE[:, 0:1],
                                        scalar1=t_mE[:, 0:1])
            t_var = gnsb.tile([2, 1], f32, tag=f"var{h}", name=f"var{h}")
            nc.vector.tensor_add(out=t_var, in0=t_mE[:, 1:2],
                                 in1=t_mE[:, 2:3])
            nc.vector.tensor_sub(out=t_var, in0=t_var, in1=t_m2)
            t_lnv = gnsb.tile([2, 1], f32, tag=f"lnv{h}", name=f"lnv{h}")
            nc.scalar.activation(out=t_lnv, in_=t_var, func=AF.Ln, bias=t_eps)
            t_im = gnsb.tile([2, 2], f32, tag=f"im{h}", name=f"im{h}")
            nc.scalar.activation(out=t_im[:, 0:1], in_=t_lnv, func=AF.Exp,
                                 scale=-0.5)
            nc.vector.tensor_copy(out=t_im[:, 1:2], in_=t_mE[:, 0:1])
            nc.tensor.matmul(ps_gn[:, 4:6], t_gselT, t_im,
                             start=True, stop=True)
            t_A = gnsb.tile([DH, 1], f32, tag=f"A{h}", name=f"A{h}")
            nc.vector.tensor_mul(out=t_A, in0=ps_gn[:, 4:5],
                                 in1=t_gs4[:, h:h + 1])
            t_t1 = gnsb.tile([DH, 1], f32, tag=f"t1_{h}", name=f"t1_{h}")
            nc.vector.tensor_mul(out=t_t1, in0=ps_gn[:, 5:6], in1=t_A)
            t_B = gnsb.tile([DH, 1], f32, tag=f"B{h}", name=f"B{h}")
            nc.vector.tensor_sub(out=t_B, in0=t_gb4[:, h:h + 1], in1=t_t1)
            stg = stgpool.tile([DH, S], fp16, tag="stg", name=f"stg{h}")
            nc.vector.tensor_scalar(
                out=stg, in0=t_y[h],
                scalar1=t_A, scalar2=t_B, op0=OP.mult, op1=OP.add)
            nc.sync.dma_start(out=d_out.ap()[h * DH:(h + 1) * DH, :], in_=stg)

        # interleaved emission: per-engine FIFO order == emission order
        S_block(0, 0, 4)
        S_block(1, 0, 1)
        D_block(0)
        S_block(1, 1, 4)
        S_block(2, 0, 1)
        D_block(1)
        P_block(0, 0, 4)
        SQ_block(0, 0)
        P_block(0, 4, 8)
        S_block(2, 1, 4)
        SQ_block(0, 1)
        S_block(3, 0, 1)
        D_block(2)
        P_block(1, 0, 4)
        SQ_block(1, 0)
        G_block(0)
        S_block(3, 1, 2)
        P_block(1, 4, 8)
        SQ_block(1, 1)
        S_block(3, 2, 4)
        P_block(2, 0, 4)
        SQ_block(2, 0)
        D_block(3)
        P_block(2, 4, 8)
        SQ_block(2, 1)
        G_block(1)
        P_block(3, 0, 4)
        SQ_block(3, 0)
        P_block(3, 4, 8)
        SQ_block(3, 1)
        G_block(2)
        G_block(3)

    _split_multiwaits(nc)
    return nc


def _prepare_in_maps(x, text_emb, attention_mask, Wq_w, Wq_b, Wk_w, Wk_b,
                     Wv_w, Wv_b, gn_scale, gn_bias):
    f32 = np.float32
    fp16 = np.float16
    x = np.asarray(x)
    text_emb = np.asarray(text_emb)
    attention_mask = np.asarray(attention_mask)
    Wq_w = np.asarray(Wq_w)
    Wq_b = np.asarray(Wq_b)
    Wk_w = np.asarray(Wk_w)
    Wk_b = np.asarray(Wk_b)
    Wv_w = np.asarray(Wv_w)
    Wv_b = np.asarray(Wv_b)
    gn_scale = np.asarray(gn_scale)
    gn_bias = np.asarray(gn_bias)

    wkT = np.ascontiguousarray(Wk_w.T).reshape(6, DH, C)
    wvT = np.ascontiguousarray(Wv_w.T).reshape(6, DH, C)
    wkp = np.ascontiguousarray(wkT.transpose(1, 0, 2).reshape(DH, 6 * CH)
                               ).astype(fp16)
    wvp = np.ascontiguousarray(wvT.transpose(1, 0, 2).reshape(DH, 6 * CH)
                               ).astype(fp16)

    # wq[d, (h*4+t)*128 + c] = Wq[128h + d, 128t + c]; last 4 cols = Wq_b
    wqr = Wq_w.reshape(NUM_HEADS, DH, 4, DH)
    wq = np.empty((DH, 16 * DH + 4), fp16)
    wq[:, 0:16 * DH] = wqr.transpose(1, 0, 2, 3).reshape(DH, 16 * DH)
    wq[:, 16 * DH:] = Wq_b.reshape(4, DH).T

    kvb = np.empty((1, 2 * C), fp16)
    kvb[0, 0:C] = Wk_b
    kvb[0, C:2 * C] = Wv_b

    gpk = np.zeros((DH, 10), f32)
    gpk[:, 0:4] = gn_scale.reshape(4, DH).T
    gpk[:, 4:8] = gn_bias.reshape(4, DH).T
    gpk[0:64, 8] = 1.0
    gpk[64:128, 9] = 1.0
    gselT = np.ascontiguousarray(gpk[:, 8:10].T)

    ones77 = np.ones((1, L), fp16)

    xp = np.ascontiguousarray(
        x.reshape(B, 4, DH, S).transpose(0, 2, 1, 3).reshape(B, DH, 4 * S)
    ).astype(fp16)
    ttp = np.ascontiguousarray(
        text_emb.transpose(0, 2, 1).reshape(B, 6, DH, L)
        .transpose(0, 2, 1, 3).reshape(B, DH, 6 * L)).astype(fp16)

    in_maps = []
    for b in range(N_CORES):
        maskf = attention_mask[b].astype(f32)
        lpk = np.zeros((L, 141), fp16)
        for j in range(NCHUNK):
            lpk[:, j * NCHUNK + j] = maskf
        lpk[:, 64:141] = np.eye(L, dtype=fp16)
        in_maps.append({
            "xp": xp[b],
            "ttp": ttp[b],
            "wkp": wkp, "wvp": wvp,
            "wq": wq,
            "lpk": lpk,
            "maskf": maskf.reshape(L, 1),
            "ones77": ones77,
            "kvb": kvb,
            "gpk": gpk,
            "gselT": gselT,
        })
    return in_maps


def kernel(**inputs):
    global _compiled
    from concourse import bass_utils

    in_maps = _prepare_in_maps(**inputs)
    if _compiled is None:
        _compiled = _build_nc()
    res = bass_utils.run_bass_kernel_spmd(
        _compiled, in_maps, core_ids=list(range(N_CORES)))
    out = np.stack([np.asarray(res.results[b]["out"]).astype(np.float32)
                    .reshape(C, H, W) for b in range(N_CORES)])
    return out


# revision 28
# speedup vs baseline: 1.1492x; 1.0147x over previous
"""Trainium2 Bass kernel: cross-attention (4 heads, image->text) + GroupNorm.

Shapes (hardcoded): x [8, 512, 64, 64] f32, text_emb [8, 77, 768] f32,
attention_mask [8, 77] i32, Wq [512, 512], Wk/Wv [512, 768], biases [512],
gn_scale/bias [512]. Output [8, 512, 64, 64] f32.

Strategy: data-parallel over batch, one batch element per NeuronCore (8 cores).
Channels-on-partitions layout [C, S], S = H*W = 4096; fp16 16-bit dtype
everywhere on the PE/DVE paths (f32 PSUM accumulation).

Key restructure vs a direct translation: the Q projection is folded into the
score matmul via associativity:
    scores_h^T = Kh @ Qh^T = (Kh @ Wq_h) @ x = M_h @ x
so the big [C,S] Q tensor (and its PSUM->SBUF copies) never exists. M_h^T
([512, 77] per head) is computed on-chip from K^T (PE transpose) and Wq.
The Q-bias term folds into the exp() bias column (per-partition ACT bias).

Pipeline per head h (j = 8 chunks of 512 pixels):
  scores:  ps_s[77,512] = sum_t mstat[h,t]^T @ x_t[:,chunk]   (PE, fp16)
  exp:     eu = exp(SCALE*ps_s + SCALE*c_h)  ACT, PSUM->SBUF fp16
  denom:   ps_d[8,512] accumulates row j = maskf . eu(h,j)    (PE, mask8 trick)
  recip:   rb = exp(-ln(ps_d))                                 (ACT)
  bcast:   rb -> DRAM row [1,4096] -> one partition-broadcast DMA
           -> rbb[128,4096] (DMA is the only partition replicator)
  PV:      ps_u[128,512] = Vm_h^T @ eu                         (PE)
  norm:    y = ps_u * rbb                                      (DVE TT)
  stats:   sum(y) via DVE tensor_reduce, sum(y^2) via ACT Square
           with accum_out, per head                            (DVE+ACT)
  GN:      group sums via tiny matmuls, istd = exp(-0.5 ln(var+eps)),
           y*A + B -> fp16 staging -> one DMA per head (host upcasts)

All DRAM inputs are host-packed into [128, *] row-contiguous blocks so each
dma_start lowers to ~128 descriptors (descriptor count, not bytes, dominated
the DMA queues in earlier versions).
"""

import os
import numpy as np

NUM_HEADS = 4
GROUPS = 8
EPS = 1e-5
B, C, H, W = 8, 512, 64, 64
S = H * W          # 4096
L, E = 77, 768
DH = C // NUM_HEADS  # 128
N_CORES = 8
NCHUNK = 8         # S chunks of 512
CH = S // NCHUNK   # 512
SCALE = DH ** -0.5
GN_P = 64          # partitions per group
GN_N = float(GN_P * S)  # elements per group

_compiled = None


def _patch_tile_drain():
    """This container's walrus rejects multi-sem-wait Drain instructions
    ("Too many sync wait commands"); split the TileContext exit drain's waits
    into single-wait instructions, which lower like raw-bass waits."""
    import concourse.tile as tile
    import concourse.mybir as mybir
    from concourse.tile import ScopedClock

    if getattr(tile.TileContext, "_drain_patched", False):
        return

    def _patched(self, tick_clock, wait_clock):
        nc = self.nc
        probe = nc.sync.nop(nofuse=True, hint="drain_wait_probe")
        wait_clock.add_sem_waits(
            probe.ins, ScopedClock({None: tick_clock.global_clock})
        )
        si = probe.ins.sync_info
        waits = list(si.on_wait) if si is not None else []
        probe.ins.sync_info = mybir.SyncInfo(on_wait=[], on_update=[])

        popped = nc._tile_sem_poison_stack.pop()
        assert popped is self._sem_poison
        assert self.sems is not None
        allocated = self.sems.allocated()
        by_id = {h.num: h for h in allocated.values()}
        for wv in waits:
            h = by_id.get(wv.id)
            assert h is not None, f"no semaphore handle for wait {wv}"
            assert wv.wait_mode == "sem-ge-imm", wv
            nc.sync.wait_ge(h, wv.wait_value)

        nc.sync.drain()
        nc.all_engine_barrier()
        nc.clear_and_free_semaphores(list(allocated.values()))
        nc.all_engine_barrier()

    tile.TileContext._drain_and_barrier = _patched
    tile.TileContext._drain_patched = True


def _patch_ldw_opt():
    """Optionally flip walrus --enable-ldw-opt (env BASS_LDW_OPT=1)."""
    if os.environ.get("BASS_LDW_OPT") != "1":
        return
    import concourse.bass_utils as bu
    if getattr(bu, "_ldw_patched", False):
        return
    orig = bu.run_command

    def patched(cmd, *a, **kw):
        cmd = ["--enable-ldw-opt=true" if c == "--enable-ldw-opt=false" else c
               for c in cmd]
        return orig(cmd, *a, **kw)

    bu.run_command = patched
    bu._ldw_patched = True


def _split_multiwaits(nc):
    """This walrus build rejects instructions carrying more than one sem wait
    ("Too many sync wait commands"). Hoist all-but-one wait of each such
    instruction onto standalone event-semaphore waits (built by the real bass
    builders so they lower correctly), inserted just before it."""
    import concourse.mybir as mybir

    eng_map = {
        mybir.EngineType.DVE: nc.vector,
        mybir.EngineType.Activation: nc.scalar,
        mybir.EngineType.PE: nc.tensor,
        mybir.EngineType.Pool: nc.gpsimd,
        mybir.EngineType.SP: nc.sync,
    }
    jobs = []
    for f in nc.m.functions:
        for bb in f.blocks:
            for inst in bb.instructions:
                si = inst.sync_info
                if si is not None and len(si.on_wait) > 1:
                    jobs.append((bb, inst))
    if not jobs:
        return 0

    tail_bb = None
    made = []
    with nc.semaphore() as dummy:
        for bb, inst in jobs:
            waits = list(inst.sync_info.on_wait)
            for w in waits[:-1]:
                bi = eng_map[inst.engine].wait_ge(dummy, 0)
                bi.ins.sync_info = mybir.SyncInfo(on_wait=[w], on_update=[])
                made.append(bi.ins)
    made_names = {m.name for m in made}
    for f in nc.m.functions:
        for bb in f.blocks:
            if any(i.name in made_names for i in bb.instructions):
                tail_bb = bb
                tail_bb.instructions = [
                    i for i in bb.instructions if i.name not in made_names]
    assert tail_bb is not None

    it = iter(made)
    n_split = 0
    for f in nc.m.functions:
        for bb in f.blocks:
            out = []
            for inst in bb.instructions:
                si = inst.sync_info
                waits = list(si.on_wait) if si is not None else []
                if len(waits) > 1 and any(inst is j[1] for j in jobs):
                    for w in waits[:-1]:
                        ev = next(it)
                        out.append(ev)
                        n_split += 1
                    inst.sync_info = mybir.SyncInfo(
                        on_wait=[waits[-1]], on_update=list(si.on_update))
                out.append(inst)
            bb.instructions = out
    return n_split


def _build_nc():
    import concourse.bass as bass
    import concourse.tile as tile
    import concourse.mybir as mybir

    _patch_tile_drain()
    _patch_ldw_opt()
    dt = mybir.dt
    f32, fp16 = dt.float32, dt.float16
    AF = mybir.ActivationFunctionType
    OP = mybir.AluOpType
    AX = mybir.AxisListType

    nc = bass.Bass()

    # ---- DRAM I/O (host-packed, row-contiguous [128, *] blocks) ----
    d_xp = nc.dram_tensor("xp", [DH, 4 * S], fp16, kind="ExternalInput")
    d_ttp = nc.dram_tensor("ttp", [DH, 6 * L], fp16, kind="ExternalInput")
    d_wkp = nc.dram_tensor("wkp", [DH, 6 * CH], fp16, kind="ExternalInput")
    d_wvp = nc.dram_tensor("wvp", [DH, 6 * CH], fp16, kind="ExternalInput")
    d_wq = nc.dram_tensor("wq", [DH, 16 * DH + 4], fp16, kind="ExternalInput")
    d_lpk = nc.dram_tensor("lpk", [L, 141], fp16, kind="ExternalInput")
    d_maskf = nc.dram_tensor("maskf", [L, 1], f32, kind="ExternalInput")
    d_ones77 = nc.dram_tensor("ones77", [1, L], fp16, kind="ExternalInput")
    d_kvb = nc.dram_tensor("kvb", [1, 2 * C], fp16, kind="ExternalInput")
    d_gpk = nc.dram_tensor("gpk", [DH, 10], f32, kind="ExternalInput")
    d_gselT = nc.dram_tensor("gselT", [2, DH], f32, kind="ExternalInput")
    d_out = nc.dram_tensor("out", [C, S], fp16, kind="ExternalOutput")
    d_rbscr = nc.dram_tensor("rbscr", [NUM_HEADS, S], fp16, kind="Internal")

    ET = 6        # k tiles for E=768
    KT = 4        # c_in tiles for C=512

    from contextlib import ExitStack

    with tile.TileContext(nc) as tc, ExitStack() as stack:
        cpool = stack.enter_context(tc.tile_pool(name="const", bufs=1))
        t_lpk = cpool.tile([L, 141], fp16, tag="lpk")
        nc.sync.dma_start(out=t_lpk, in_=d_lpk.ap())
        t_maskf = cpool.tile([L, 1], f32, tag="maskf")
        nc.sync.dma_start(out=t_maskf, in_=d_maskf.ap())
        t_ones77 = cpool.tile([1, L], fp16, tag="ones77")
        nc.sync.dma_start(out=t_ones77, in_=d_ones77.ap())
        t_gpk = cpool.tile([DH, 10], f32, tag="gpk")
        nc.sync.dma_start(out=t_gpk, in_=d_gpk.ap())
        t_gselT = cpool.tile([2, DH], f32, tag="gselT")
        nc.sync.dma_start(out=t_gselT, in_=d_gselT.ap())
        t_wq = cpool.tile([DH, 16 * DH + 4], fp16, tag="wq")
        nc.sync.dma_start(out=t_wq, in_=d_wq.ap())
        t_eps = cpool.tile([2, 1], f32, tag="eps")
        nc.vector.memset(t_eps, EPS)

        t_mask8 = t_lpk[:, 0:64]
        t_ident = t_lpk[:, 64:141]
        t_gs4 = t_gpk[:, 0:4]
        t_gb4 = t_gpk[:, 4:8]
        t_gsel = t_gpk[:, 8:10]

        # persistent state
        t_kht = cpool.tile([DH, NUM_HEADS * L], fp16, tag="kht")
        t_vm = cpool.tile([L, C], fp16, tag="vm")
        t_mstat = cpool.tile([DH, 16 * L], fp16, tag="mstat")
        t_c4 = cpool.tile([L, 4], f32, tag="c4")
        t_eu = cpool.tile([L, NUM_HEADS * S], fp16, tag="eu")
        t_y = [cpool.tile([DH, S], fp16, tag=f"y{h}", name=f"y{h}")
               for h in range(NUM_HEADS)]
        t_rb = [cpool.tile([NCHUNK, CH], fp16, tag=f"rb{h}", name=f"rb{h}")
                for h in range(NUM_HEADS)]
        t_sq = cpool.tile([DH, S], fp16, tag="sq")
        t_syc = [cpool.tile([DH, NCHUNK], f32, tag=f"syc{h}", name=f"syc{h}")
                 for h in range(NUM_HEADS)]
        t_sy2 = [cpool.tile([DH, 2], f32, tag=f"sy2{h}", name=f"sy2{h}")
                 for h in range(NUM_HEADS)]
        t_xt = [cpool.tile([DH, S], fp16, tag=f"xt{t}", name=f"xt{t}")
                for t in range(KT)]

        # ---------- phase 0: K/V projections, K^T, M_h^T, c_h ----------
        with (
            tc.tile_pool(name="kvw", bufs=1) as kvw,
            tc.tile_pool(name="kvps", bufs=2, space="PSUM") as kvps,
        ):
            t_ttp = kvw.tile([DH, 6 * L], fp16, tag="ttp")
            nc.sync.dma_start(out=t_ttp, in_=d_ttp.ap())
            t_wkp = kvw.tile([DH, 6 * CH], fp16, tag="wkp")
            nc.sync.dma_start(out=t_wkp, in_=d_wkp.ap())
            t_kvb = kvw.tile([1, 2 * C], fp16, tag="kvb")
            nc.sync.dma_start(out=t_kvb, in_=d_kvb.ap())
            t_wvp = kvw.tile([DH, 6 * CH], fp16, tag="wvp")
            nc.sync.dma_start(out=t_wvp, in_=d_wvp.ap())

            # x DMAs after the phase-0 weights (HWDGE ring is FIFO; weights
            # gate the first matmuls)
            for t in range(KT):
                nc.sync.dma_start(out=t_xt[t][:, 0:1024],
                                  in_=d_xp.ap()[:, t * S:t * S + 1024])
            for t in range(KT):
                nc.sync.dma_start(out=t_xt[t][:, 1024:S],
                                  in_=d_xp.ap()[:, t * S + 1024:(t + 1) * S])

            t_ksb = kvw.tile([L, C], fp16, tag="ksb")
            ps_k = kvps.tile([L, C], f32, tag="pskv")
            for k in range(ET):
                nc.tensor.matmul(ps_k, t_ttp[:, k * L:(k + 1) * L],
                                 t_wkp[:, k * CH:(k + 1) * CH],
                                 start=(k == 0), stop=False)
            nc.tensor.matmul(ps_k, t_ones77, t_kvb[:, 0:C],
                             start=False, stop=True)
            nc.vector.tensor_copy(out=t_ksb, in_=ps_k)

            ps_v = kvps.tile([L, C], f32, tag="pskv")
            for k in range(ET):
                nc.tensor.matmul(ps_v, t_ttp[:, k * L:(k + 1) * L],
                                 t_wvp[:, k * CH:(k + 1) * CH],
                                 start=(k == 0), stop=False)
            nc.tensor.matmul(ps_v, t_ones77, t_kvb[:, C:2 * C],
                             start=False, stop=True)
            nc.vector.tensor_scalar_mul(out=t_vm, in0=ps_v, scalar1=t_maskf)

            # K^T per head via PE transpose
            for h in range(NUM_HEADS):
                ps_t = kvps.tile([DH, L], fp16, tag="pstr")
                nc.tensor.transpose(ps_t, t_ksb[:, h * DH:(h + 1) * DH],
                                    t_ident)
                nc.vector.tensor_copy(out=t_kht[:, h * L:(h + 1) * L],
                                      in_=ps_t)

            # c_h = Kh @ bq_h  -> exp bias column (times SCALE)
            ps_c4 = kvps.tile([L, 4], f32, tag="psc4")
            for h in range(NUM_HEADS):
                nc.tensor.matmul(ps_c4[:, h:h + 1],
                                 t_kht[:, h * L:(h + 1) * L],
                                 t_wq[:, 16 * DH + h:16 * DH + h + 1],
                                 start=True, stop=True)
            nc.vector.tensor_scalar_mul(out=t_c4, in0=ps_c4, scalar1=SCALE)

            # M_h^T tiles: [c_tile 128, 77] per (h, t)
            for h in range(NUM_HEADS):
                ps_m = kvps.tile([DH, 4 * L], f32, tag="psm", name=f"psm{h}")
                for t in range(KT):
                    nc.tensor.matmul(
                        ps_m[:, t * L:(t + 1) * L],
                        t_wq[:, (h * 4 + t) * DH:(h * 4 + t + 1) * DH],
                        t_kht[:, h * L:(h + 1) * L],
                        start=True, stop=True)
                nc.vector.tensor_copy(
                    out=t_mstat[:, h * 4 * L:(h + 1) * 4 * L], in_=ps_m)

        # ---------- main pipeline ----------
        spool = stack.enter_context(
            tc.tile_pool(name="spool", bufs=2, space="PSUM"))
        dpool = stack.enter_context(
            tc.tile_pool(name="dpool", bufs=1, space="PSUM"))
        upool = stack.enter_context(
            tc.tile_pool(name="upool", bufs=2, space="PSUM"))
        gnps = stack.enter_context(
            tc.tile_pool(name="gnps", bufs=1, space="PSUM"))
        rbbpool = stack.enter_context(tc.tile_pool(name="rbb", bufs=2))
        lnpool = stack.enter_context(tc.tile_pool(name="lnp", bufs=2))
        stgpool = stack.enter_context(tc.tile_pool(name="stg", bufs=2))
        gnsb = stack.enter_context(tc.tile_pool(name="gnsb", bufs=1))

        rbb_tiles = {}

        def S_block(h, jp0, jp1):
            for jp in range(jp0, jp1):
                ps_s = spool.tile([L, 2 * CH], f32, tag="pss",
                                  name=f"pss{h}_{jp}")
                for u in range(2):
                    for t in range(KT):
                        nc.tensor.matmul(
                            ps_s[:, u * CH:(u + 1) * CH],
                            t_mstat[:, (h * 4 + t) * L:(h * 4 + t + 1) * L],
                            t_xt[t][:, (2 * jp + u) * CH:(2 * jp + u + 1) * CH],
                            start=(t == 0), stop=(t == KT - 1))
                nc.scalar.activation(
                    out=t_eu[:, (h * NCHUNK + 2 * jp) * CH:
                             (h * NCHUNK + 2 * jp + 2) * CH],
                    in_=ps_s, func=AF.Exp, scale=SCALE,
                    bias=t_c4[:, h:h + 1])

        def D_block(h):
            ps_d = dpool.tile([NCHUNK, CH], f32, tag="psd", name=f"psd{h}")
            for j in range(NCHUNK):
                nc.tensor.matmul(
                    ps_d,
                    t_mask8[:, j * NCHUNK:(j + 1) * NCHUNK],
                    t_eu[:, (h * NCHUNK + j) * CH:(h * NCHUNK + j + 1) * CH],
                    start=(j == 0), stop=(j == NCHUNK - 1))
            t_ln = lnpool.tile([NCHUNK, CH], f32, tag="lnd", name=f"lnd{h}")
            nc.scalar.activation(out=t_ln, in_=ps_d, func=AF.Ln)
            nc.scalar.activation(out=t_rb[h], in_=t_ln, func=AF.Exp,
                                 scale=-1.0)
            nc.sync.dma_start(
                out=d_rbscr.ap()[h:h + 1, :]
                .rearrange("r (p f) -> (r p) f", p=NCHUNK),
                in_=t_rb[h])
            rbb = rbbpool.tile([DH, S], fp16, tag="rbb", name=f"rbb{h}")
            nc.sync.dma_start(
                out=rbb.rearrange("p (one f) -> p one f", one=1),
                in_=d_rbscr.ap()[h:h + 1, :].partition_broadcast(DH))
            rbb_tiles[h] = rbb

        def P_block(h, j0, j1):
            for j in range(j0, j1):
                ps_u = upool.tile([DH, CH], f32, tag="psu",
                                  name=f"psu{h}_{j}")
                nc.tensor.matmul(
                    ps_u,
                    t_vm[:, h * DH:(h + 1) * DH],
                    t_eu[:, (h * NCHUNK + j) * CH:(h * NCHUNK + j + 1) * CH],
                    start=True, stop=True)
                nc.vector.scalar_tensor_tensor(
                    out=t_y[h][:, j * CH:(j + 1) * CH], in0=ps_u,
                    scalar=1.0,
                    in1=rbb_tiles[h][:, j * CH:(j + 1) * CH],
                    op0=OP.mult, op1=OP.mult,
                    accum_out=t_syc[h][:, j:j + 1])


        def SQ_block(h, half):
            hs = S // 2
            nc.scalar.activation(
                out=t_sq[:, half * hs:(half + 1) * hs],
                in_=t_y[h][:, half * hs:(half + 1) * hs], func=AF.Square,
                accum_out=t_sy2[h][:, half:half + 1])

        st3s = {}

        def G_stats(h):
            st3 = gnsb.tile([DH, 3], f32, tag=f"st3{h}", name=f"st3{h}")
            nc.vector.tensor_reduce(out=st3[:, 0:1], in_=t_syc[h],
                                    axis=AX.X, op=OP.add)
            nc.vector.tensor_copy(out=st3[:, 1:3], in_=t_sy2[h])
            st3s[h] = st3

        def G_block(h):
            st3 = st3s[h]
            ps_gn = gnps.tile([DH, 8], f32, tag="psgn", name=f"psgn{h}")
            nc.tensor.matmul(ps_gn[0:2, 0:3], t_gsel, st3,
                             start=True, stop=True)
            t_mE = gnsb.tile([2, 3], f32, tag=f"mE{h}", name=f"mE{h}")
            nc.vector.tensor_scalar_mul(out=t_mE, in0=ps_gn[0:2, 0:3],
                                        scalar1=1.0 / GN_N)
            t_m2 = gnsb.tile([2, 1], f32, tag=f"m2_{h}", name=f"m2_{h}")
            nc.vector.tensor_scalar_mul(out=t_m2, in0=t_mE[:, 0:1],
                                        scalar1=t_mE[:, 0:1])
            t_var = gnsb.tile([2, 1], f32, tag=f"var{h}", name=f"var{h}")
            nc.vector.tensor_add(out=t_var, in0=t_mE[:, 1:2],
                                 in1=t_mE[:, 2:3])
            nc.vector.tensor_sub(out=t_var, in0=t_var, in1=t_m2)
            t_lnv = gnsb.tile([2, 1], f32, tag=f"lnv{h}", name=f"lnv{h}")
            nc.scalar.activation(out=t_lnv, in_=t_var, func=AF.Ln, bias=t_eps)
            t_im = gnsb.tile([2, 2], f32, tag=f"im{h}", name=f"im{h}")
            nc.scalar.activation(out=t_im[:, 0:1], in_=t_lnv, func=AF.Exp,
                                 scale=-0.5)
            nc.vector.tensor_copy(out=t_im[:, 1:2], in_=t_mE[:, 0:1])
            nc.tensor.matmul(ps_gn[:, 4:6], t_gselT, t_im,
                             start=True, stop=True)
            t_A = gnsb.tile([DH, 1], f32, tag=f"A{h}", name=f"A{h}")
            nc.vector.tensor_mul(out=t_A, in0=ps_gn[:, 4:5],
                                 in1=t_gs4[:, h:h + 1])
            t_t1 = gnsb.tile([DH, 1], f32, tag=f"t1_{h}", name=f"t1_{h}")
            nc.vector.tensor_mul(out=t_t1, in0=ps_gn[:, 5:6], in1=t_A)
            t_B = gnsb.tile([DH, 1], f32, tag=f"B{h}", name=f"B{h}")
            nc.vector.tensor_sub(out=t_B, in0=t_gb4[:, h:h + 1], in1=t_t1)
            stg = stgpool.tile([DH, S], fp16, tag="stg", name=f"stg{h}")
            nc.vector.tensor_scalar(
                out=stg, in0=t_y[h],
                scalar1=t_A, scalar2=t_B, op0=OP.mult, op1=OP.add)
            nc.sync.dma_start(out=d_out.ap()[h * DH:(h + 1) * DH, :], in_=stg)

        # interleaved emission: per-engine FIFO order == emission order
        S_block(0, 0, 4)
        S_block(1, 0, 1)
        D_block(0)
        S_block(1, 1, 4)
        S_block(2, 0, 1)
        D_block(1)
        P_block(0, 0, 4)
        SQ_block(0, 0)
        P_block(0, 4, 8)
        S_block(2, 1, 4)
        SQ_block(0, 1)
        S_block(3, 0, 1)
        D_block(2)
        P_block(1, 0, 4)
        SQ_block(1, 0)
        G_stats(0)
        G_block(0)
        S_block(3, 1, 2)
        P_block(1, 4, 8)
        SQ_block(1, 1)
        G_stats(1)
        S_block(3, 2, 4)
        P_block(2, 0, 4)
        SQ_block(2, 0)
        D_block(3)
        P_block(2, 4, 8)
        SQ_block(2, 1)
        G_stats(2)
        G_block(1)
        P_block(3, 0, 4)
        SQ_block(3, 0)
        G_block(2)
        P_block(3, 4, 8)
        SQ_block(3, 1)
        G_stats(3)
        G_block(3)

    _split_multiwaits(nc)
    return nc


def _prepare_in_maps(x, text_emb, attention_mask, Wq_w, Wq_b, Wk_w, Wk_b,
                     Wv_w, Wv_b, gn_scale, gn_bias):
    f32 = np.float32
    fp16 = np.float16
    x = np.asarray(x)
    text_emb = np.asarray(text_emb)
    attention_mask = np.asarray(attention_mask)
    Wq_w = np.asarray(Wq_w)
    Wq_b = np.asarray(Wq_b)
    Wk_w = np.asarray(Wk_w)
    Wk_b = np.asarray(Wk_b)
    Wv_w = np.asarray(Wv_w)
    Wv_b = np.asarray(Wv_b)
    gn_scale = np.asarray(gn_scale)
    gn_bias = np.asarray(gn_bias)

    wkT = np.ascontiguousarray(Wk_w.T).reshape(6, DH, C)
    wvT = np.ascontiguousarray(Wv_w.T).reshape(6, DH, C)
    wkp = np.ascontiguousarray(wkT.transpose(1, 0, 2).reshape(DH, 6 * CH)
                               ).astype(fp16)
    wvp = np.ascontiguousarray(wvT.transpose(1, 0, 2).reshape(DH, 6 * CH)
                               ).astype(fp16)

    # wq[d, (h*4+t)*128 + c] = Wq[128h + d, 128t + c]; last 4 cols = Wq_b
    wqr = Wq_w.reshape(NUM_HEADS, DH, 4, DH)
    wq = np.empty((DH, 16 * DH + 4), fp16)
    wq[:, 0:16 * DH] = wqr.transpose(1, 0, 2, 3).reshape(DH, 16 * DH)
    wq[:, 16 * DH:] = Wq_b.reshape(4, DH).T

    kvb = np.empty((1, 2 * C), fp16)
    kvb[0, 0:C] = Wk_b
    kvb[0, C:2 * C] = Wv_b

    gpk = np.zeros((DH, 10), f32)
    gpk[:, 0:4] = gn_scale.reshape(4, DH).T
    gpk[:, 4:8] = gn_bias.reshape(4, DH).T
    gpk[0:64, 8] = 1.0
    gpk[64:128, 9] = 1.0
    gselT = np.ascontiguousarray(gpk[:, 8:10].T)

    ones77 = np.ones((1, L), fp16)

    xp = np.ascontiguousarray(
        x.reshape(B, 4, DH, S).transpose(0, 2, 1, 3).reshape(B, DH, 4 * S)
    ).astype(fp16)
    ttp = np.ascontiguousarray(
        text_emb.transpose(0, 2, 1).reshape(B, 6, DH, L)
        .transpose(0, 2, 1, 3).reshape(B, DH, 6 * L)).astype(fp16)

    in_maps = []
    for b in range(N_CORES):
        maskf = attention_mask[b].astype(f32)
        lpk = np.zeros((L, 141), fp16)
        for j in range(NCHUNK):
            lpk[:, j * NCHUNK + j] = maskf
        lpk[:, 64:141] = np.eye(L, dtype=fp16)
        in_maps.append({
            "xp": xp[b],
            "ttp": ttp[b],
            "wkp": wkp, "wvp": wvp,
            "wq": wq,
            "lpk": lpk,
            "maskf": maskf.reshape(L, 1),
            "ones77": ones77,
            "kvb": kvb,
            "gpk": gpk,
            "gselT": gselT,
        })
    return in_maps


def kernel(**inputs):
    global _compiled
    from concourse import bass_utils

    in_maps = _prepare_in_maps(**inputs)
    if _compiled is None:
        _compiled = _build_nc()
    res = bass_utils.run_bass_kernel_spmd(
        _compiled, in_maps, core_ids=list(range(N_CORES)))
    out = np.stack([np.asarray(res.results[b]["out"]).astype(np.float32)
                    .reshape(C, H, W) for b in range(N_CORES)])
    return out
